# revision 1
# baseline (speedup 1.0000x reference)
"""Multi-head attention (B=4, S=2048, D=1024, H=16) + output projection on 8 trn2 cores.

Sharding: no collectives. Core c handles batch c//2, query rows (c%2)*1024..+1024,
all 16 heads. Each core needs full K/V for its batch; W_out/b_out replicated.
The per-core output block [1024, 1024] is the final projected output for those
query rows, so the host just concatenates.

Per-core algorithm (all matmuls bf16, fp32 PSUM accumulation):
  - q, k, W cast fp32->bf16 via SWDGE DMA into DRAM scratch, then HWDGE
    DMA-transpose loads: qT/kT/WT tiles with head_dim (d) on partitions.
  - per head-pair hp (2 heads stacked on 128 partitions):
      per j-chunk jc (16 x 128 keys):
        scoresT[j, i] matmuls, 2 heads row-packed via tile_position (0,0)/(64,0)
        -> psum [128, 2048] = [headA 1024 i | headB 1024 i]
        one ScalarE Exp over the 4-bank psum (scale=1/8 folded in) -> SBUF bf16
        AV matmuls: lhsT = v_aug [128 j, 65] (ones column -> softmax sums for
        free), accumulate over jc into psum [65, 1024] per head
      normalize: reciprocal of sums row, gpsimd partition_broadcast, DVE mult
      -> attT tiles [128 d, 1024 i] bf16 (odd head partition-shifted via DMA)
  - projection: final[i, e] = attT.T @ WT accumulated over the 8 d-chunks,
    bias added on DVE from a partition-broadcast bias tile, fp32 out.
"""

import numpy as np

import concourse.bass as bass
import concourse.tile as tile
from concourse import bacc, mybir
from concourse.bass_utils import run_bass_kernel_spmd

B = 4
S = 2048
DM = 1024
H = 16
DK = 64
SCALE = DK**-0.5
I = 1024  # local query rows per core
NJC = S // 128  # 16 j-chunks
NHP = H // 2  # 8 head pairs == 8 d-chunks of the model dim

F32 = mybir.dt.float32
BF16 = mybir.dt.bfloat16


def build(nc: bass.Bass):
    q = nc.dram_tensor("q", [I, DM], F32, kind="ExternalInput").ap()
    k = nc.dram_tensor("k", [S, DM], F32, kind="ExternalInput").ap()
    v = nc.dram_tensor("v", [S, DM], F32, kind="ExternalInput").ap()
    w = nc.dram_tensor("w", [DM, DM], F32, kind="ExternalInput").ap()
    b = nc.dram_tensor("b", [DM], F32, kind="ExternalInput").ap()
    out = nc.dram_tensor("out", [I, DM], F32, kind="ExternalOutput").ap()

    q_bf = nc.dram_tensor("q_bf", [I, DM], BF16).ap()
    k_bf = nc.dram_tensor("k_bf", [S, DM], BF16).ap()
    w_bf = nc.dram_tensor("w_bf", [DM, DM], BF16).ap()

    with tile.TileContext(nc) as tc:
        with (
            tc.tile_pool(name="persist", bufs=1) as pers,
            tc.tile_pool(name="expp", bufs=3) as expp,
            tc.tile_pool(name="nrmp", bufs=2) as nrmp,
            tc.tile_pool(name="finp", bufs=2) as finp,
        ):
            # ---- prelude: casts + transposed loads + v/bias loads ----
            nc.gpsimd.dma_start(out=q_bf[:, :], in_=q[:, :])
            nc.gpsimd.dma_start(out=k_bf[:, :], in_=k[:, :])
            nc.gpsimd.dma_start(out=w_bf[:, :], in_=w[:, :])

            qT, kT, vA, attT, wT = [], [], [], [], []
            for hp in range(NHP):
                qt = pers.tile([128, I], BF16, name=f"qT{hp}", tag=f"qT{hp}")
                nc.sync.dma_start(
                    out=qt[:, :], in_=q_bf[:, hp * 128 : (hp + 1) * 128], transpose=True
                )
                qT.append(qt)
                kt = pers.tile([128, S], BF16, name=f"kT{hp}", tag=f"kT{hp}")
                nc.sync.dma_start(
                    out=kt[:, :], in_=k_bf[:, hp * 128 : (hp + 1) * 128], transpose=True
                )
                kT.append(kt)

            for h in range(H):
                va = pers.tile([128, NJC * 65], BF16, name=f"vA{h}", tag=f"vA{h}")
                src = v[:, h * DK : (h + 1) * DK].rearrange(
                    "(jc p) d -> p jc d", p=128
                )
                dst = va[:, :].rearrange("p (jc e) -> p jc e", e=65)
                nc.gpsimd.dma_start(out=dst[:, :, 0:DK], in_=src)
                nc.vector.memset(dst[:, :, DK], 1.0)
                vA.append(va)

            for dc in range(NHP):
                wt = pers.tile([128, DM], BF16, name=f"wT{dc}", tag=f"wT{dc}")
                nc.sync.dma_start(
                    out=wt[:, :], in_=w_bf[:, dc * 128 : (dc + 1) * 128], transpose=True
                )
                wT.append(wt)

            bias_sb = pers.tile([1, DM], F32, name="bias_sb", tag="bias_sb")
            nc.sync.dma_start(out=bias_sb[:, :], in_=b[None, :])
            bias_bc = pers.tile([128, DM], F32, name="bias_bc", tag="bias_bc")
            nc.gpsimd.partition_broadcast(bias_bc[:, :], bias_sb[0:1, :])

            for hp in range(NHP):
                at = pers.tile([128, I], BF16, name=f"attT{hp}", tag=f"attT{hp}")
                attT.append(at)

            # ---- attention (ACT-bound pipeline) ----
            with (
                tc.tile_pool(name="smmp", bufs=1, space="PSUM") as smmp,
                tc.tile_pool(name="avp", bufs=2, space="PSUM") as avp,
            ):
                for hp in range(NHP):
                    av = [
                        avp.tile([65, I], F32, name=f"av{hp}_{h2}", tag="av")
                        for h2 in range(2)
                    ]
                    for jc in range(NJC):
                        smm = smmp.tile([128, 2 * I], F32, name=f"smm{hp}_{jc}", tag="smm")
                        for h2 in range(2):
                            lhsT = kT[hp][
                                h2 * DK : (h2 + 1) * DK, jc * 128 : (jc + 1) * 128
                            ]
                            for ih in range(2):
                                nc.tensor.matmul(
                                    smm[:, h2 * I + ih * 512 : h2 * I + (ih + 1) * 512],
                                    lhsT,
                                    qT[hp][
                                        h2 * DK : (h2 + 1) * DK,
                                        ih * 512 : (ih + 1) * 512,
                                    ],
                                    start=True,
                                    stop=True,
                                    tile_position=(h2 * DK, 0),
                                )
                        expt = expp.tile(
                            [128, 2 * I], BF16, name=f"expt{hp}_{jc}", tag="expt"
                        )
                        nc.scalar.activation(
                            expt[:, :],
                            smm[:, :],
                            mybir.ActivationFunctionType.Exp,
                            scale=SCALE,
                        )
                        for h2 in range(2):
                            h = 2 * hp + h2
                            for ih in range(2):
                                nc.tensor.matmul(
                                    av[h2][:, ih * 512 : (ih + 1) * 512],
                                    vA[h][:, jc * 65 : jc * 65 + 65],
                                    expt[:, h2 * I + ih * 512 : h2 * I + (ih + 1) * 512],
                                    start=(jc == 0),
                                    stop=(jc == NJC - 1),
                                    skip_group_check=True,
                                )
                    # normalize into attT
                    for h2 in range(2):
                        recip = nrmp.tile([1, I], F32, name=f"rc{hp}_{h2}", tag="recip")
                        nc.vector.reciprocal(recip[:, :], av[h2][DK : DK + 1, :])
                        rb = nrmp.tile([DK, I], F32, name=f"rb{hp}_{h2}", tag="rb")
                        nc.gpsimd.partition_broadcast(rb[:, :], recip[0:1, :])
                        if h2 == 0:
                            nc.vector.tensor_mul(
                                attT[hp][0:DK, :], av[h2][0:DK, :], rb[:, :]
                            )
                        else:
                            nrm = nrmp.tile([DK, I], BF16, name=f"nm{hp}", tag="nrm")
                            nc.vector.tensor_mul(nrm[:, :], av[h2][0:DK, :], rb[:, :])
                            nc.sync.dma_start(out=attT[hp][DK:128, :], in_=nrm[:, :])

            # ---- output projection ----
            with tc.tile_pool(name="projp", bufs=2, space="PSUM") as projp:
                for ic in range(I // 128):
                    for ec in range(2):
                        pp = projp.tile([128, 512], F32, name=f"pp{ic}_{ec}", tag="pp")
                        for dc in range(NHP):
                            nc.tensor.matmul(
                                pp[:, :],
                                attT[dc][:, ic * 128 : (ic + 1) * 128],
                                wT[dc][:, ec * 512 : (ec + 1) * 512],
                                start=(dc == 0),
                                stop=(dc == NHP - 1),
                                skip_group_check=True,
                            )
                        fin = finp.tile([128, 512], F32, name=f"fin{ic}_{ec}", tag="fin")
                        nc.vector.tensor_add(
                            fin[:, :], pp[:, :], bias_bc[:, ec * 512 : (ec + 1) * 512]
                        )
                        nc.sync.dma_start(
                            out=out[
                                ic * 128 : (ic + 1) * 128, ec * 512 : (ec + 1) * 512
                            ],
                            in_=fin[:, :],
                        )
    return nc


_NC_CACHE = {}


def _get_nc():
    if "nc" not in _NC_CACHE:
        nc = bacc.Bacc("TRN2", target_bir_lowering=False, debug=False)
        build(nc)
        nc.compile()
        _NC_CACHE["nc"] = nc
    return _NC_CACHE["nc"]


def kernel(q, k, v, W_out, b_out, _trace=False, _trace_kwargs=None):
    q = np.asarray(q, dtype=np.float32)
    k = np.asarray(k, dtype=np.float32)
    v = np.asarray(v, dtype=np.float32)
    W_out = np.ascontiguousarray(np.asarray(W_out, dtype=np.float32))
    b_out = np.ascontiguousarray(np.asarray(b_out, dtype=np.float32))

    nc = _get_nc()
    in_maps = []
    for c in range(8):
        bi, half = c // 2, c % 2
        in_maps.append(
            {
                "q": np.ascontiguousarray(q[bi, half * I : (half + 1) * I, :]),
                "k": np.ascontiguousarray(k[bi]),
                "v": np.ascontiguousarray(v[bi]),
                "w": W_out,
                "b": b_out,
            }
        )
    res = run_bass_kernel_spmd(
        nc,
        in_maps,
        core_ids=list(range(8)),
        trace=_trace,
        **(_trace_kwargs or {}),
    )
    out = np.empty((B, S, DM), np.float32)
    for c in range(8):
        bi, half = c // 2, c % 2
        out[bi, half * I : (half + 1) * I, :] = res.results[c]["out"]
    if _trace:
        return out, res
    return out


# revision 3
# speedup vs baseline: 1.5572x; 1.5572x over previous
"""Multi-head attention (B=4, S=2048, D=1024, H=16) + output projection on 8 trn2 cores.

Sharding: no collectives. Core c handles batch c//2, query rows (c%2)*1024..+1024,
all 16 heads. Each core needs full K/V for its batch; W_out/b_out replicated.
The per-core output block [1024, 1024] is the final projected output for those
query rows, so the host just concatenates.

Per-core algorithm (all matmuls bf16, fp32 PSUM accumulation):
  - q, k, W cast fp32->bf16 via SWDGE DMA (per-128-column chunks so the first
    head pair is ready early) into DRAM scratch, then HWDGE DMA-transpose
    loads: qT/kT/WT tiles with head_dim (d) on partitions.
  - per head-pair hp (2 heads stacked on 128 partitions):
      per j-chunk jc (16 x 128 keys), per head h2:
        scoresT[j, i] matmul into psum [128, 1024] (double-buffered pool, so
        PE runs a jc ahead of ScalarE), 2 heads row-packed via tile_position
        ScalarE Exp over the 2-bank psum (scale=1/8 folded in) -> SBUF bf16
        AV matmuls: lhsT = v_aug [128 j, 65] (ones column -> softmax sums for
        free), accumulate over jc into psum [65, 1024] per head
      one DVE copy psum->SBUF releases the AV accumulator; the normalization
      (fast reciprocal of the sums row, gpsimd partition_broadcast, DVE mult)
      trails off the critical path into attT [128 d, 1024 i] bf16
  - projection: final[i, e] = attT.T @ WT accumulated over the 8 d-chunks,
    bias added on DVE from a partition-broadcast bias tile, fp32 out.
"""

import numpy as np

import concourse.bass as bass
import concourse.tile as tile
from concourse import bacc, mybir
from concourse.bass_utils import run_bass_kernel_spmd

B = 4
S = 2048
DM = 1024
H = 16
DK = 64
SCALE = DK**-0.5
I = 1024  # local query rows per core
NJC = S // 128  # 16 j-chunks
NHP = H // 2  # 8 head pairs == 8 d-chunks of the model dim

F32 = mybir.dt.float32
BF16 = mybir.dt.bfloat16


def build(nc: bass.Bass):
    q = nc.dram_tensor("q", [I, DM], F32, kind="ExternalInput").ap()
    k = nc.dram_tensor("k", [S, DM], F32, kind="ExternalInput").ap()
    v = nc.dram_tensor("v", [S, DM], F32, kind="ExternalInput").ap()
    w = nc.dram_tensor("w", [DM, DM], F32, kind="ExternalInput").ap()
    b = nc.dram_tensor("b", [DM], F32, kind="ExternalInput").ap()
    out = nc.dram_tensor("out", [I, DM], F32, kind="ExternalOutput").ap()

    q_bf = nc.dram_tensor("q_bf", [I, DM], BF16).ap()
    k_bf = nc.dram_tensor("k_bf", [S, DM], BF16).ap()
    w_bf = nc.dram_tensor("w_bf", [DM, DM], BF16).ap()

    with tile.TileContext(nc) as tc:
        with (
            tc.tile_pool(name="persist", bufs=1) as pers,
            tc.tile_pool(name="expp", bufs=4) as expp,
            tc.tile_pool(name="avsbp", bufs=4) as avsbp,
            tc.tile_pool(name="nrmp", bufs=2) as nrmp,
            tc.tile_pool(name="finp", bufs=2) as finp,
        ):
            # ---- PE warmup: dummy matmuls so HAM un-throttles during the
            # DMA prelude (zeroed input; results never read) ----
            warm_sb = pers.tile([128, 512], BF16, name="warm_sb", tag="warm_sb")
            nc.vector.memset(warm_sb[:, :], 0.0)

            # ---- prelude: per-column-chunk casts + transposed loads ----
            qT, kT, vA, attT, wT = [], [], [], [], []
            for hp in range(NHP):
                sl = slice(hp * 128, (hp + 1) * 128)
                nc.gpsimd.dma_start(out=q_bf[:, sl], in_=q[:, sl])
                qt = pers.tile([128, I], BF16, name=f"qT{hp}", tag=f"qT{hp}")
                nc.sync.dma_start(out=qt[:, :], in_=q_bf[:, sl], transpose=True)
                qT.append(qt)
                nc.gpsimd.dma_start(out=k_bf[:, sl], in_=k[:, sl])
                kt = pers.tile([128, S], BF16, name=f"kT{hp}", tag=f"kT{hp}")
                nc.sync.dma_start(out=kt[:, :], in_=k_bf[:, sl], transpose=True)
                kT.append(kt)
                # v for the two heads of this pair
                for h2 in range(2):
                    h = 2 * hp + h2
                    va = pers.tile([128, NJC * 65], BF16, name=f"vA{h}", tag=f"vA{h}")
                    src = v[:, h * DK : (h + 1) * DK].rearrange(
                        "(jc p) d -> p jc d", p=128
                    )
                    dst = va[:, :].rearrange("p (jc e) -> p jc e", e=65)
                    nc.gpsimd.dma_start(out=dst[:, :, 0:DK], in_=src)
                    nc.vector.memset(dst[:, :, DK], 1.0)
                    vA.append(va)
                at = pers.tile([128, I], BF16, name=f"attT{hp}", tag=f"attT{hp}")
                attT.append(at)

            # warmup matmuls (no data deps beyond the memset)
            with tc.tile_pool(name="warmp", bufs=1, space="PSUM") as warmp:
                wps = warmp.tile([128, 512], F32, name="wps", tag="wps")
                for _ in range(14):
                    nc.tensor.matmul(
                        wps[:, :],
                        warm_sb[:, 0:128],
                        warm_sb[:, :],
                        start=True,
                        stop=True,
                        skip_group_check=True,
                    )

            # W/bias prep (needed only for the projection epilogue)
            nc.gpsimd.dma_start(out=w_bf[:, :], in_=w[:, :])
            for dc in range(NHP):
                wt = pers.tile([128, DM], BF16, name=f"wT{dc}", tag=f"wT{dc}")
                nc.sync.dma_start(
                    out=wt[:, :], in_=w_bf[:, dc * 128 : (dc + 1) * 128], transpose=True
                )
                wT.append(wt)
            bias_sb = pers.tile([1, DM], F32, name="bias_sb", tag="bias_sb")
            nc.sync.dma_start(out=bias_sb[:, :], in_=b[None, :])
            bias_bc = pers.tile([128, DM], F32, name="bias_bc", tag="bias_bc")
            nc.gpsimd.partition_broadcast(bias_bc[:, :], bias_sb[0:1, :])

            # ---- attention (ScalarE-bound pipeline) ----
            with (
                tc.tile_pool(name="smmp", bufs=2, space="PSUM") as smmp,
                tc.tile_pool(name="avp", bufs=2, space="PSUM") as avp,
            ):
                for hp in range(NHP):
                    av = [
                        avp.tile([65, I], F32, name=f"av{hp}_{h2}", tag="av")
                        for h2 in range(2)
                    ]
                    for jc in range(NJC):
                        for h2 in range(2):
                            smm = smmp.tile(
                                [128, I], F32, name=f"smm{hp}_{jc}_{h2}", tag="smm"
                            )
                            lhsT = kT[hp][
                                h2 * DK : (h2 + 1) * DK, jc * 128 : (jc + 1) * 128
                            ]
                            for ih in range(2):
                                nc.tensor.matmul(
                                    smm[:, ih * 512 : (ih + 1) * 512],
                                    lhsT,
                                    qT[hp][
                                        h2 * DK : (h2 + 1) * DK,
                                        ih * 512 : (ih + 1) * 512,
                                    ],
                                    start=True,
                                    stop=True,
                                    tile_position=(h2 * DK, 0),
                                )
                            expt = expp.tile(
                                [128, I], BF16, name=f"ex{hp}_{jc}_{h2}", tag="expt"
                            )
                            nc.scalar.activation(
                                expt[:, :],
                                smm[:, :],
                                mybir.ActivationFunctionType.Exp,
                                scale=SCALE,
                            )
                            h = 2 * hp + h2
                            for ih in range(2):
                                nc.tensor.matmul(
                                    av[h2][:, ih * 512 : (ih + 1) * 512],
                                    vA[h][:, jc * 65 : jc * 65 + 65],
                                    expt[:, ih * 512 : (ih + 1) * 512],
                                    start=(jc == 0),
                                    stop=(jc == NJC - 1),
                                    skip_group_check=True,
                                )
                    # release psum fast: one copy to SBUF, then normalize there
                    for h2 in range(2):
                        avsb = avsbp.tile(
                            [DK, I], F32, name=f"avsb{hp}_{h2}", tag="avsb"
                        )
                        nc.vector.tensor_copy(avsb[:, :], av[h2][0:DK, :])
                        sums = nrmp.tile([1, I], F32, name=f"sm{hp}_{h2}", tag="sums")
                        nc.vector.tensor_copy(sums[:, :], av[h2][DK : DK + 1, :])
                        rc = nrmp.tile([1, I], F32, name=f"rc{hp}_{h2}", tag="rc")
                        nc.vector.reciprocal_approx_fast(rc[:, :], sums[:, :])
                        rb = nrmp.tile([DK, I], F32, name=f"rb{hp}_{h2}", tag="rb")
                        nc.gpsimd.partition_broadcast(rb[:, :], rc[0:1, :])
                        nc.vector.tensor_mul(
                            attT[hp][h2 * DK : (h2 + 1) * DK, :],
                            avsb[:, :],
                            rb[:, :],
                        )

            # ---- output projection ----
            with tc.tile_pool(name="projp", bufs=4, space="PSUM") as projp:
                for ic in range(I // 128):
                    for ec in range(2):
                        pp = projp.tile([128, 512], F32, name=f"pp{ic}_{ec}", tag="pp")
                        for dc in range(NHP):
                            nc.tensor.matmul(
                                pp[:, :],
                                attT[dc][:, ic * 128 : (ic + 1) * 128],
                                wT[dc][:, ec * 512 : (ec + 1) * 512],
                                start=(dc == 0),
                                stop=(dc == NHP - 1),
                                skip_group_check=True,
                            )
                        fin = finp.tile([128, 512], F32, name=f"fin{ic}_{ec}", tag="fin")
                        nc.vector.tensor_add(
                            fin[:, :], pp[:, :], bias_bc[:, ec * 512 : (ec + 1) * 512]
                        )
                        nc.sync.dma_start(
                            out=out[
                                ic * 128 : (ic + 1) * 128, ec * 512 : (ec + 1) * 512
                            ],
                            in_=fin[:, :],
                        )
    return nc


_NC_CACHE = {}


def _get_nc():
    if "nc" not in _NC_CACHE:
        nc = bacc.Bacc("TRN2", target_bir_lowering=False, debug=False)
        build(nc)
        nc.compile()
        _NC_CACHE["nc"] = nc
    return _NC_CACHE["nc"]


def kernel(q, k, v, W_out, b_out, _trace=False, _trace_kwargs=None):
    q = np.asarray(q, dtype=np.float32)
    k = np.asarray(k, dtype=np.float32)
    v = np.asarray(v, dtype=np.float32)
    W_out = np.ascontiguousarray(np.asarray(W_out, dtype=np.float32))
    b_out = np.ascontiguousarray(np.asarray(b_out, dtype=np.float32))

    nc = _get_nc()
    in_maps = []
    for c in range(8):
        bi, half = c // 2, c % 2
        in_maps.append(
            {
                "q": np.ascontiguousarray(q[bi, half * I : (half + 1) * I, :]),
                "k": np.ascontiguousarray(k[bi]),
                "v": np.ascontiguousarray(v[bi]),
                "w": W_out,
                "b": b_out,
            }
        )
    res = run_bass_kernel_spmd(
        nc,
        in_maps,
        core_ids=list(range(8)),
        trace=_trace,
        **(_trace_kwargs or {}),
    )
    out = np.empty((B, S, DM), np.float32)
    for c in range(8):
        bi, half = c // 2, c % 2
        out[bi, half * I : (half + 1) * I, :] = res.results[c]["out"]
    if _trace:
        return out, res
    return out


# revision 6
# speedup vs baseline: 1.5684x; 1.0072x over previous
"""Multi-head attention (B=4, S=2048, D=1024, H=16) + output projection on 8 trn2 cores.

Sharding: no collectives. Core c handles batch c//2, query rows (c%2)*1024..+1024,
all 16 heads. Each core needs full K/V for its batch; W_out/b_out replicated.
The per-core output block [1024, 1024] is the final projected output for those
query rows, so the host just concatenates.

Per-core algorithm (all matmuls bf16, fp32 PSUM accumulation):
  - q, k, W cast fp32->bf16 via SWDGE DMA (per-128-column chunks so the first
    head pair is ready early) into DRAM scratch, then HWDGE DMA-transpose
    loads: qT/kT/WT tiles with head_dim (d) on partitions.
  - per head-pair hp (2 heads stacked on 128 partitions):
      per j-chunk jc (16 x 128 keys), per head h2:
        scoresT[j, i] matmul into psum [128, 1024] (double-buffered pool, so
        PE runs a jc ahead of ScalarE), 2 heads row-packed via tile_position
        ScalarE Exp over the 2-bank psum (scale=1/8 folded in) -> SBUF bf16
        AV matmuls: lhsT = v_aug [128 j, 65] (ones column -> softmax sums for
        free), accumulate over jc into psum [65, 1024] per head
      one DVE copy psum->SBUF releases the AV accumulator; the normalization
      (fast reciprocal of the sums row, gpsimd partition_broadcast, DVE mult)
      trails off the critical path into attT [128 d, 1024 i] bf16
  - projection: final[i, e] = attT.T @ WT accumulated over the 8 d-chunks,
    bias added on DVE from a partition-broadcast bias tile, fp32 out.
"""

import numpy as np

import concourse.bass as bass
import concourse.tile as tile
from concourse import bacc, mybir
from concourse.bass_utils import run_bass_kernel_spmd

B = 4
S = 2048
DM = 1024
H = 16
DK = 64
SCALE = DK**-0.5
I = 1024  # local query rows per core
NJC = S // 128  # 16 j-chunks
NHP = H // 2  # 8 head pairs == 8 d-chunks of the model dim

F32 = mybir.dt.float32
BF16 = mybir.dt.bfloat16


def build(nc: bass.Bass):
    q = nc.dram_tensor("q", [I, DM], F32, kind="ExternalInput").ap()
    k = nc.dram_tensor("k", [S, DM], F32, kind="ExternalInput").ap()
    v = nc.dram_tensor("v", [S, DM], F32, kind="ExternalInput").ap()
    w = nc.dram_tensor("w", [DM, DM], F32, kind="ExternalInput").ap()
    b = nc.dram_tensor("b", [DM], F32, kind="ExternalInput").ap()
    out = nc.dram_tensor("out", [I, DM], F32, kind="ExternalOutput").ap()

    # separate scratch tensors per head-pair: avoids false whole-tensor
    # WAR deps between the next chunk's cast-write and this chunk's
    # transpose-read, and makes both cast dst and transpose src contiguous
    q_bf = [nc.dram_tensor(f"q_bf{i}", [I, 128], BF16).ap() for i in range(NHP)]
    k_bf = [nc.dram_tensor(f"k_bf{i}", [S, 128], BF16).ap() for i in range(NHP)]
    w_bf = [nc.dram_tensor(f"w_bf{i}", [DM, 128], BF16).ap() for i in range(NHP)]

    with tile.TileContext(nc) as tc:
        with (
            tc.tile_pool(name="persist", bufs=1) as pers,
            tc.tile_pool(name="expp", bufs=4) as expp,
            tc.tile_pool(name="avsbp", bufs=4) as avsbp,
            tc.tile_pool(name="nrmp", bufs=2) as nrmp,
            tc.tile_pool(name="finp", bufs=2) as finp,
        ):
            # ---- PE warmup: dummy matmuls so HAM un-throttles during the
            # DMA prelude (zeroed input; results never read) ----
            warm_sb = pers.tile([128, 512], BF16, name="warm_sb", tag="warm_sb")
            nc.vector.memset(warm_sb[:, :], 0.0)

            # ---- prelude: per-column-chunk casts + transposed loads ----
            qT, kT, vA, attT, wT = [], [], [], [], []
            # casts + v loads on the SWDGE queue, interleaved per head pair so
            # early pairs complete first
            for hp in range(NHP):
                sl = slice(hp * 128, (hp + 1) * 128)
                nc.gpsimd.dma_start(out=q_bf[hp][:, :], in_=q[:, sl])
                nc.gpsimd.dma_start(out=k_bf[hp][:, :], in_=k[:, sl])
                for h2 in range(2):
                    h = 2 * hp + h2
                    va = pers.tile([128, NJC * 65], BF16, name=f"vA{h}", tag=f"vA{h}")
                    src = v[:, h * DK : (h + 1) * DK].rearrange(
                        "(jc p) d -> p jc d", p=128
                    )
                    dst = va[:, :].rearrange("p (jc e) -> p jc e", e=65)
                    nc.gpsimd.dma_start(out=dst[:, :, 0:DK], in_=src)
                    nc.vector.memset(dst[:, :, DK], 1.0)
                    vA.append(va)
            # transpose loads on the HWDGE(SP) queue; each waits only on its
            # own chunk's cast
            for hp in range(NHP):
                qt = pers.tile([128, I], BF16, name=f"qT{hp}", tag=f"qT{hp}")
                nc.sync.dma_start(out=qt[:, :], in_=q_bf[hp][:, :], transpose=True)
                qT.append(qt)
                kt = pers.tile([128, S], BF16, name=f"kT{hp}", tag=f"kT{hp}")
                nc.sync.dma_start(out=kt[:, :], in_=k_bf[hp][:, :], transpose=True)
                kT.append(kt)
                at = pers.tile([128, I], BF16, name=f"attT{hp}", tag=f"attT{hp}")
                attT.append(at)

            # warmup matmuls (no data deps beyond the memset)
            with tc.tile_pool(name="warmp", bufs=1, space="PSUM") as warmp:
                wps = warmp.tile([128, 512], F32, name="wps", tag="wps")
                for _ in range(14):
                    nc.tensor.matmul(
                        wps[:, :],
                        warm_sb[:, 0:128],
                        warm_sb[:, :],
                        start=True,
                        stop=True,
                        skip_group_check=True,
                    )

            # W/bias prep (needed only for the projection epilogue)
            for dc in range(NHP):
                nc.gpsimd.dma_start(
                    out=w_bf[dc][:, :], in_=w[:, dc * 128 : (dc + 1) * 128]
                )
            for dc in range(NHP):
                wt = pers.tile([128, DM], BF16, name=f"wT{dc}", tag=f"wT{dc}")
                nc.sync.dma_start(out=wt[:, :], in_=w_bf[dc][:, :], transpose=True)
                wT.append(wt)
            bias_sb = pers.tile([1, DM], F32, name="bias_sb", tag="bias_sb")
            nc.sync.dma_start(out=bias_sb[:, :], in_=b[None, :])
            bias_bc = pers.tile([128, DM], F32, name="bias_bc", tag="bias_bc")
            nc.gpsimd.partition_broadcast(bias_bc[:, :], bias_sb[0:1, :])

            # ---- attention (ScalarE-bound pipeline) ----
            with (
                tc.tile_pool(name="smmp", bufs=2, space="PSUM") as smmp,
                tc.tile_pool(name="avp", bufs=2, space="PSUM") as avp,
            ):
                for hp in range(NHP):
                    av = [
                        avp.tile([65, I], F32, name=f"av{hp}_{h2}", tag="av")
                        for h2 in range(2)
                    ]
                    for jc in range(NJC):
                        for h2 in range(2):
                            smm = smmp.tile(
                                [128, I], F32, name=f"smm{hp}_{jc}_{h2}", tag="smm"
                            )
                            lhsT = kT[hp][
                                h2 * DK : (h2 + 1) * DK, jc * 128 : (jc + 1) * 128
                            ]
                            for ih in range(2):
                                nc.tensor.matmul(
                                    smm[:, ih * 512 : (ih + 1) * 512],
                                    lhsT,
                                    qT[hp][
                                        h2 * DK : (h2 + 1) * DK,
                                        ih * 512 : (ih + 1) * 512,
                                    ],
                                    start=True,
                                    stop=True,
                                    tile_position=(h2 * DK, 0),
                                )
                            expt = expp.tile(
                                [128, I], BF16, name=f"ex{hp}_{jc}_{h2}", tag="expt"
                            )
                            nc.scalar.activation(
                                expt[:, :],
                                smm[:, :],
                                mybir.ActivationFunctionType.Exp,
                                scale=SCALE,
                            )
                            h = 2 * hp + h2
                            for ih in range(2):
                                nc.tensor.matmul(
                                    av[h2][:, ih * 512 : (ih + 1) * 512],
                                    vA[h][:, jc * 65 : jc * 65 + 65],
                                    expt[:, ih * 512 : (ih + 1) * 512],
                                    start=(jc == 0),
                                    stop=(jc == NJC - 1),
                                    skip_group_check=True,
                                )
                    # release psum fast: one copy to SBUF, then normalize there
                    for h2 in range(2):
                        avsb = avsbp.tile(
                            [DK, I], F32, name=f"avsb{hp}_{h2}", tag="avsb"
                        )
                        nc.vector.tensor_copy(avsb[:, :], av[h2][0:DK, :])
                        sums = nrmp.tile([1, I], F32, name=f"sm{hp}_{h2}", tag="sums")
                        nc.vector.tensor_copy(sums[:, :], av[h2][DK : DK + 1, :])
                        rc = nrmp.tile([1, I], F32, name=f"rc{hp}_{h2}", tag="rc")
                        nc.vector.reciprocal_approx_fast(rc[:, :], sums[:, :])
                        rb = nrmp.tile([DK, I], F32, name=f"rb{hp}_{h2}", tag="rb")
                        nc.gpsimd.partition_broadcast(rb[:, :], rc[0:1, :])
                        nc.vector.tensor_mul(
                            attT[hp][h2 * DK : (h2 + 1) * DK, :],
                            avsb[:, :],
                            rb[:, :],
                        )

            # ---- output projection ----
            with tc.tile_pool(name="projp", bufs=4, space="PSUM") as projp:
                for ic in range(I // 128):
                    for ec in range(2):
                        pp = projp.tile([128, 512], F32, name=f"pp{ic}_{ec}", tag="pp")
                        for dc in range(NHP):
                            nc.tensor.matmul(
                                pp[:, :],
                                attT[dc][:, ic * 128 : (ic + 1) * 128],
                                wT[dc][:, ec * 512 : (ec + 1) * 512],
                                start=(dc == 0),
                                stop=(dc == NHP - 1),
                                skip_group_check=True,
                            )
                        fin = finp.tile([128, 512], F32, name=f"fin{ic}_{ec}", tag="fin")
                        nc.vector.tensor_add(
                            fin[:, :], pp[:, :], bias_bc[:, ec * 512 : (ec + 1) * 512]
                        )
                        nc.sync.dma_start(
                            out=out[
                                ic * 128 : (ic + 1) * 128, ec * 512 : (ec + 1) * 512
                            ],
                            in_=fin[:, :],
                        )
    return nc


_NC_CACHE = {}


def _get_nc():
    if "nc" not in _NC_CACHE:
        nc = bacc.Bacc("TRN2", target_bir_lowering=False, debug=False)
        build(nc)
        nc.compile()
        _NC_CACHE["nc"] = nc
    return _NC_CACHE["nc"]


def kernel(q, k, v, W_out, b_out, _trace=False, _trace_kwargs=None):
    q = np.asarray(q, dtype=np.float32)
    k = np.asarray(k, dtype=np.float32)
    v = np.asarray(v, dtype=np.float32)
    W_out = np.ascontiguousarray(np.asarray(W_out, dtype=np.float32))
    b_out = np.ascontiguousarray(np.asarray(b_out, dtype=np.float32))

    nc = _get_nc()
    in_maps = []
    for c in range(8):
        bi, half = c // 2, c % 2
        in_maps.append(
            {
                "q": np.ascontiguousarray(q[bi, half * I : (half + 1) * I, :]),
                "k": np.ascontiguousarray(k[bi]),
                "v": np.ascontiguousarray(v[bi]),
                "w": W_out,
                "b": b_out,
            }
        )
    res = run_bass_kernel_spmd(
        nc,
        in_maps,
        core_ids=list(range(8)),
        trace=_trace,
        **(_trace_kwargs or {}),
    )
    out = np.empty((B, S, DM), np.float32)
    for c in range(8):
        bi, half = c // 2, c % 2
        out[bi, half * I : (half + 1) * I, :] = res.results[c]["out"]
    if _trace:
        return out, res
    return out
